# revision 3
# baseline (speedup 1.0000x reference)
"""Distributed Trainium2 Bass kernel for ArcticAttention (GQA + RoPE + sliding window).

Sharding: tensor-parallel over heads across 8 cores. Core c owns q heads
4c..4c+3 and kv head c (exactly one GQA group). Per core:
  - q/k/v projections (bf16 matmuls, fp32 PSUM) producing qT/kT [dh, tok]
    and v [tok, dh] layouts,
  - RoPE fused on the vector engine from host-precomputed cos/sign-folded-sin
    tables,
  - sliding-window attention in S^T = K@Q^T layout (softmax over the
    partition axis via a ones-vector matmul; 1/l broadcast via gpsimd
    partition_broadcast; PV matmul needs no transposes anywhere),
  - AllGather of ctx^T features (bf16, 2 MB/core per batch),
  - column-sharded o_proj producing out^T [oc, tok]; host concatenates.
"""

import os
import sys

sys.path.insert(0, "/opt/pypackages")
sys.path.insert(0, "/opt/trn_rl_repo")

import numpy as np
import ml_dtypes

BF16 = ml_dtypes.bfloat16

B, S, HID = 2, 2048, 4096
H, HKV, DH = 32, 8, 128
G = H // HKV
WIN = 1024
THETA = 10000.0
NCORES = 8
HPC = H // NCORES          # 4 q heads per core
BT = B * S                 # 4096 tokens
TOKB = 256                 # projection/o_proj token block
QB = 512                   # attention query block
NQB = S // QB              # 4 query blocks per batch
SCALE = 1.0 / float(np.sqrt(DH))

# r = kt - (Q0 - 8); masked (non-trivially) for these relative key tiles
MASK_RS = (0, 1, 2, 3, 8, 9, 10, 11)
MSLOT = {r: i for i, r in enumerate(MASK_RS)}


def _span(r):
    """Live query range [qlo, qhi) within a 512-wide q block for relative
    key tile r (keys at absolute tile Q0-8+r)."""
    qlo = max(0, (r - 8) * 128)
    qhi = min(QB, (r - 8) * 128 + 1024 + 127)
    return qlo, qhi


def _build_nc():
    import concourse.bass as bass
    import concourse.bacc as bacc
    import concourse.mybir as mybir
    from concourse import tile

    dt = mybir.dt
    bf = dt.bfloat16
    f32 = dt.float32
    AF = mybir.ActivationFunctionType

    nc = bacc.Bacc(
        "TRN2",
        target_bir_lowering=False,
        debug=False,
        enable_asserts=False,
        num_devices=NCORES,
    )

    hiddenT = nc.dram_tensor("hiddenT", [HID, BT], bf, kind="ExternalInput")
    wq = nc.dram_tensor("wq", [HID, HPC * DH], bf, kind="ExternalInput")
    wk = nc.dram_tensor("wk", [HID, DH], bf, kind="ExternalInput")
    wv = nc.dram_tensor("wv", [HID, DH], bf, kind="ExternalInput")
    wo = nc.dram_tensor("wo", [HID, HPC * DH], bf, kind="ExternalInput")
    cost = nc.dram_tensor("cost", [DH, S], f32, kind="ExternalInput")
    sinm = nc.dram_tensor("sinm", [DH, S], f32, kind="ExternalInput")
    maskt = nc.dram_tensor("maskt", [len(MASK_RS) * 128, QB], bf, kind="ExternalInput")
    outT = nc.dram_tensor("outT", [HPC * DH, BT], f32, kind="ExternalOutput")

    NA = HID // 128  # 32 hid chunks

    with tile.TileContext(nc) as tc:
        with (
            tc.tile_pool(name="const", bufs=1) as cpool,
            tc.tile_pool(name="hid", bufs=2) as hidpool,
            tc.tile_pool(name="kv", bufs=2) as kvpool,
            tc.tile_pool(name="qt", bufs=10) as qtpool,
            tc.tile_pool(name="work", bufs=2) as wpool,
            tc.tile_pool(name="pt", bufs=3) as ptpool,
            tc.tile_pool(name="mm", bufs=3, space="PSUM") as mmpool,
            tc.tile_pool(name="sps", bufs=2, space="PSUM") as spool,
            tc.tile_pool(name="ctxps", bufs=2, space="PSUM") as cxpool,
            tc.tile_pool(name="lps", bufs=1, space="PSUM") as lpool,
            tc.tile_pool(name="dram", bufs=1, space="DRAM") as dpool,
        ):
            # ---- resident constants ----
            wq_sb = cpool.tile([128, NA * HPC * DH], bf, tag="wq")
            wk_sb = cpool.tile([128, NA * DH], bf, tag="wk")
            wv_sb = cpool.tile([128, NA * DH], bf, tag="wv")
            wo_sb = cpool.tile([128, NA * HPC * DH], bf, tag="wo")
            cos_sb = cpool.tile([128, S], f32, tag="cos")
            sin_sb = cpool.tile([128, S], f32, tag="sin")
            mask_sb = cpool.tile([128, len(MASK_RS) * QB], bf, tag="mask")
            ones_sb = cpool.tile([128, 1], bf, tag="ones")

            for a in range(NA):
                nc.sync.dma_start(
                    wq_sb[:, a * 512 : (a + 1) * 512], wq[a * 128 : (a + 1) * 128, :]
                )
                nc.sync.dma_start(
                    wk_sb[:, a * 128 : (a + 1) * 128], wk[a * 128 : (a + 1) * 128, :]
                )
                nc.sync.dma_start(
                    wv_sb[:, a * 128 : (a + 1) * 128], wv[a * 128 : (a + 1) * 128, :]
                )
                nc.sync.dma_start(
                    wo_sb[:, a * 512 : (a + 1) * 512], wo[a * 128 : (a + 1) * 128, :]
                )
            nc.sync.dma_start(cos_sb[:], cost[:])
            nc.sync.dma_start(sin_sb[:], sinm[:])
            for i in range(len(MASK_RS)):
                nc.sync.dma_start(
                    mask_sb[:, i * QB : (i + 1) * QB], maskt[i * 128 : (i + 1) * 128, :]
                )
            nc.any.memset(ones_sb[:], 1.0)

            # per-batch DRAM bounce buffers for the collective
            ctxl = [
                dpool.tile([HPC * DH, S], bf, tag=f"ctxl{b}", name=f"ctxl{b}")
                for b in range(B)
            ]
            ctxf = [
                dpool.tile(
                    [H * DH, S], bf, tag=f"ctxf{b}", addr_space="Shared",
                    name=f"ctxf{b}",
                )
                for b in range(B)
            ]

            def rope_drain(ps, dst, tok0, n):
                """dst = ps * cos + rot_half(ps) * sin  (sign folded into sinm).
                ps: PSUM [128, n] fp32; dst: SBUF bf16 [128, n] slice.
                tok0: batch-local token offset for table slicing."""
                t1 = wpool.tile([128, TOKB], f32, tag="ropet1")
                t2 = wpool.tile([128, TOKB], f32, tag="ropet2")
                cs = cos_sb[:, tok0 : tok0 + n]
                sn = sin_sb[:, tok0 : tok0 + n]
                nc.vector.tensor_mul(t1[:, :n], ps, cs)
                nc.vector.tensor_mul(t2[0:64, :n], ps[64:128, :], sn[0:64, :])
                nc.vector.tensor_mul(t2[64:128, :n], ps[0:64, :], sn[64:128, :])
                nc.vector.tensor_add(dst, t1[:, :n], t2[:, :n])

            for b in range(B):
                # kT [dh, tok] and v [tok, dh] for this batch's kv head
                kT_sb = kvpool.tile([128, S], bf, tag="kT")
                v_sb = kvpool.tile([128, S], bf, tag="v")
                qts = {}

                for qbi in range(NQB):
                    Q0 = 4 * qbi
                    for h in range(HPC):
                        qts[(qbi, h)] = qtpool.tile(
                            [128, QB], bf, tag="qtile", name=f"qt{b}_{qbi}_{h}"
                        )
                    # ---- projections for the two 256-token blocks of this qb
                    for tbh in range(2):
                        ltok = qbi * QB + tbh * TOKB     # batch-local token
                        gtok = b * S + ltok              # global token
                        hid_sb = hidpool.tile([128, NA * TOKB], bf, tag="hid")
                        for a in range(NA):
                            nc.sync.dma_start(
                                hid_sb[:, a * TOKB : (a + 1) * TOKB],
                                hiddenT[a * 128 : (a + 1) * 128, gtok : gtok + TOKB],
                            )
                        # q heads
                        for h in range(HPC):
                            ps = mmpool.tile([128, TOKB], f32, tag="mmps")
                            for a in range(NA):
                                nc.tensor.matmul(
                                    ps[:],
                                    wq_sb[:, a * 512 + h * 128 : a * 512 + (h + 1) * 128],
                                    hid_sb[:, a * TOKB : (a + 1) * TOKB],
                                    start=(a == 0),
                                    stop=(a == NA - 1),
                                )
                            rope_drain(
                                ps[:], qts[(qbi, h)][:, tbh * TOKB : (tbh + 1) * TOKB],
                                ltok, TOKB,
                            )
                        # k head
                        ps = mmpool.tile([128, TOKB], f32, tag="mmps")
                        for a in range(NA):
                            nc.tensor.matmul(
                                ps[:],
                                wk_sb[:, a * 128 : (a + 1) * 128],
                                hid_sb[:, a * TOKB : (a + 1) * TOKB],
                                start=(a == 0),
                                stop=(a == NA - 1),
                            )
                        rope_drain(ps[:], kT_sb[:, ltok : ltok + TOKB], ltok, TOKB)
                        # v: [tok, dh] via hid as stationary
                        ps = mmpool.tile([128, TOKB], f32, tag="mmps")
                        for j in range(2):
                            for a in range(NA):
                                nc.tensor.matmul(
                                    ps[:, j * 128 : (j + 1) * 128],
                                    hid_sb[:, a * TOKB + j * 128 : a * TOKB + (j + 1) * 128],
                                    wv_sb[:, a * 128 : (a + 1) * 128],
                                    start=(a == 0),
                                    stop=(a == NA - 1),
                                )
                        nc.vector.tensor_copy(v_sb[:, ltok : ltok + TOKB], ps[:])

                    # ---- attention for this query block, all 4 heads ----
                    kts = [Q0] + [
                        kt for kt in range(max(0, Q0 - 8), Q0 + 4) if kt != Q0
                    ]
                    for h in range(HPC):
                        qt = qts.pop((qbi, h))
                        ctx_ps = cxpool.tile([128, QB], f32, tag="ctxps")
                        l_ps = lpool.tile([1, QB], f32, tag="lps")
                        for idx, kt in enumerate(kts):
                            r = kt - (Q0 - 8)
                            qlo, qhi = _span(r)
                            s_ps = spool.tile([128, QB], f32, tag="sps")
                            nc.tensor.matmul(
                                s_ps[:, qlo:qhi],
                                kT_sb[:, kt * 128 : (kt + 1) * 128],
                                qt[:, qlo:qhi],
                                start=True,
                                stop=True,
                            )
                            pt = ptpool.tile([128, QB], bf, tag="pt")
                            nc.scalar.activation(
                                pt[:, qlo:qhi], s_ps[:, qlo:qhi], AF.Exp, scale=SCALE
                            )
                            if r in MSLOT:
                                m0 = MSLOT[r] * QB
                                nc.vector.tensor_mul(
                                    pt[:, qlo:qhi],
                                    pt[:, qlo:qhi],
                                    mask_sb[:, m0 + qlo : m0 + qhi],
                                )
                            last = idx == len(kts) - 1
                            nc.tensor.matmul(
                                ctx_ps[:, qlo:qhi],
                                v_sb[:, kt * 128 : (kt + 1) * 128],
                                pt[:, qlo:qhi],
                                start=(idx == 0),
                                stop=last,
                            )
                            nc.tensor.matmul(
                                l_ps[0:1, qlo:qhi],
                                ones_sb[:, 0:1],
                                pt[:, qlo:qhi],
                                start=(idx == 0),
                                stop=last,
                            )
                        lrec = wpool.tile([1, QB], f32, tag="lrec")
                        nc.vector.reciprocal(lrec[:], l_ps[:])
                        lb = wpool.tile([128, QB], f32, tag="lb")
                        nc.gpsimd.partition_broadcast(lb[:], lrec[0:1, :])
                        ctx_sb = wpool.tile([128, QB], bf, tag="ctxsb")
                        nc.vector.tensor_mul(ctx_sb[:], ctx_ps[:], lb[:])
                        nc.sync.dma_start(
                            ctxl[b][h * 128 : (h + 1) * 128, qbi * QB : (qbi + 1) * QB],
                            ctx_sb[:],
                        )

                nc.gpsimd.collective_compute(
                    "AllGather",
                    __import__("concourse.mybir", fromlist=["AluOpType"]).AluOpType.bypass,
                    replica_groups=[list(range(NCORES))],
                    ins=[ctxl[b][:].opt()],
                    outs=[ctxf[b][:].opt()],
                )

            # ---- o_proj: outT[oc, tok] = wo[:, oc].T @ ctx_full[f, tok] ----
            for b in range(B):
                for tbo in range(S // TOKB):
                    ltok = tbo * TOKB
                    gtok = b * S + ltok
                    cf = hidpool.tile([128, NA * TOKB], bf, tag="hid")
                    for a in range(NA):
                        nc.sync.dma_start(
                            cf[:, a * TOKB : (a + 1) * TOKB],
                            ctxf[b][a * 128 : (a + 1) * 128, ltok : ltok + TOKB],
                        )
                    for oc in range(HPC):
                        ps = mmpool.tile([128, TOKB], f32, tag="mmps")
                        for a in range(NA):
                            nc.tensor.matmul(
                                ps[:],
                                wo_sb[:, a * 512 + oc * 128 : a * 512 + (oc + 1) * 128],
                                cf[:, a * TOKB : (a + 1) * TOKB],
                                start=(a == 0),
                                stop=(a == NA - 1),
                            )
                        osb = wpool.tile([128, TOKB], f32, tag="osb")
                        nc.vector.tensor_copy(osb[:], ps[:])
                        nc.sync.dma_start(
                            outT[oc * 128 : (oc + 1) * 128, gtok : gtok + TOKB],
                            osb[:],
                        )

    nc.compile()
    return nc


_NC = None


def _get_nc():
    global _NC
    if _NC is None:
        _NC = _build_nc()
    return _NC


def _prep_inputs(hidden_states, q_proj_w, k_proj_w, v_proj_w, o_proj_w, position_ids):
    hidden_states = np.asarray(hidden_states, dtype=np.float32)
    hT = np.ascontiguousarray(hidden_states.reshape(BT, HID).T).astype(BF16)

    pos = np.asarray(position_ids)[0].astype(np.float32)  # [S]
    inv = 1.0 / (THETA ** (np.arange(0, DH, 2, dtype=np.float32) / DH))  # [64]
    ang = pos[:, None] * inv[None, :]  # [S, 64]
    c = np.cos(ang).T.astype(np.float32)  # [64, S]
    s = np.sin(ang).T.astype(np.float32)
    cost = np.ascontiguousarray(np.concatenate([c, c], axis=0))  # [128, S]
    sinm = np.ascontiguousarray(np.concatenate([-s, s], axis=0))

    kj = np.arange(128)[:, None]
    qi = np.arange(QB)[None, :]
    masks = []
    for r in MASK_RS:
        d = (8 - r) * 128 + qi - kj
        masks.append(((d >= 0) & (d < WIN)).astype(np.float32))
    maskt = np.ascontiguousarray(np.concatenate(masks, axis=0)).astype(BF16)

    q_proj_w = np.asarray(q_proj_w, dtype=np.float32)
    k_proj_w = np.asarray(k_proj_w, dtype=np.float32)
    v_proj_w = np.asarray(v_proj_w, dtype=np.float32)
    o_proj_w = np.asarray(o_proj_w, dtype=np.float32)

    in_maps = []
    for core in range(NCORES):
        r0q = core * HPC * DH
        r0k = core * DH
        in_maps.append(
            {
                "hiddenT": hT,
                "wq": np.ascontiguousarray(
                    q_proj_w[r0q : r0q + HPC * DH, :].T
                ).astype(BF16),
                "wk": np.ascontiguousarray(k_proj_w[r0k : r0k + DH, :].T).astype(BF16),
                "wv": np.ascontiguousarray(v_proj_w[r0k : r0k + DH, :].T).astype(BF16),
                "wo": np.ascontiguousarray(
                    o_proj_w[r0q : r0q + HPC * DH, :].T
                ).astype(BF16),
                "cost": cost,
                "sinm": sinm,
                "maskt": maskt,
            }
        )
    return in_maps


def run(inputs, trace=False):
    from concourse.bass_utils import run_bass_kernel_spmd

    nc = _get_nc()
    in_maps = _prep_inputs(
        inputs["hidden_states"],
        inputs["q_proj_w"],
        inputs["k_proj_w"],
        inputs["v_proj_w"],
        inputs["o_proj_w"],
        inputs["position_ids"],
    )
    res = run_bass_kernel_spmd(
        nc, in_maps, core_ids=list(range(NCORES)), trace=trace
    )
    out = np.empty((BT, HID), dtype=np.float32)
    for core in range(NCORES):
        o = np.asarray(res.results[core]["outT"], dtype=np.float32)  # [512, BT]
        out[:, core * HPC * DH : (core + 1) * HPC * DH] = o.T
    return out.reshape(B, S, HID), res


def kernel(**inputs):
    out, _ = run(inputs, trace=False)
    return out


# revision 17
# speedup vs baseline: 1.1833x; 1.1833x over previous
"""Distributed Trainium2 Bass kernel for ArcticAttention (GQA + RoPE + sliding window).

Sharding: tensor-parallel over heads across 8 cores. Core c owns q heads
4c..4c+3 and kv head c (exactly one GQA group). Per core:
  - q/k/v projections (bf16 matmuls, fp32 PSUM) producing qT/kT [dh, tok]
    and v [tok, dh] layouts,
  - RoPE fused on the vector engine from host-precomputed cos/sign-folded-sin
    tables,
  - sliding-window attention in S^T = K@Q^T layout (softmax over the
    partition axis via a ones-vector matmul; 1/l broadcast via gpsimd
    partition_broadcast; PV matmul needs no transposes anywhere),
  - AllGather of ctx^T features (bf16, 1 MB/core per batch-half, 4 total,
    interleaved with compute so the wire time hides),
  - column-sharded o_proj producing out^T [oc, tok]; host concatenates.
"""

import os
import sys

sys.path.insert(0, "/opt/pypackages")
sys.path.insert(0, "/opt/trn_rl_repo")

import numpy as np
import ml_dtypes

BF16 = ml_dtypes.bfloat16

B, S, HID = 2, 2048, 4096
H, HKV, DH = 32, 8, 128
G = H // HKV
WIN = 1024
THETA = 10000.0
NCORES = 8
HPC = H // NCORES          # 4 q heads per core
BT = B * S                 # 4096 tokens
QB = 512                   # token block for projections, attention, o_proj
NQB = S // QB              # 4 blocks per batch
NA = HID // 128            # 32 hid chunks
SCALE = 1.0 / float(np.sqrt(DH))

MASK_RS = (0, 1, 2, 3, 8, 9, 10, 11)
MSLOT = {r: i for i, r in enumerate(MASK_RS)}


def _span(r):
    qlo = max(0, (r - 8) * 128)
    qhi = min(QB, (r - 8) * 128 + 1024 + 127)
    return qlo, qhi


def _build_nc():
    import concourse.bass as bass
    import concourse.bacc as bacc
    import concourse.mybir as mybir
    from concourse import tile

    dt = mybir.dt
    bf = dt.bfloat16
    f32 = dt.float32
    AF = mybir.ActivationFunctionType

    nc = bacc.Bacc(
        "TRN2",
        target_bir_lowering=False,
        debug=False,
        enable_asserts=False,
        num_devices=NCORES,
    )

    hiddenT = nc.dram_tensor("hiddenT", [HID, BT], bf, kind="ExternalInput")
    wq = nc.dram_tensor("wq", [HID, HPC * DH], bf, kind="ExternalInput")
    wk = nc.dram_tensor("wk", [HID, DH], bf, kind="ExternalInput")
    wv = nc.dram_tensor("wv", [HID, DH], bf, kind="ExternalInput")
    wo = nc.dram_tensor("wo", [HID, HPC * DH], bf, kind="ExternalInput")
    cost = nc.dram_tensor("cost", [DH, S], f32, kind="ExternalInput")
    sinm = nc.dram_tensor("sinm", [DH, S], f32, kind="ExternalInput")
    maskt = nc.dram_tensor("maskt", [len(MASK_RS) * 128, QB], bf, kind="ExternalInput")
    outT = nc.dram_tensor("outT", [HPC * DH, BT], f32, kind="ExternalOutput")
    dbg = nc.dram_tensor("dbg", [5 * 128, QB], bf, kind="ExternalOutput")

    hidden3 = hiddenT[:].rearrange("(a p) t -> p a t", p=128)
    wq3 = wq[:].rearrange("(a p) d -> p a d", p=128)
    wk3 = wk[:].rearrange("(a p) d -> p a d", p=128)
    wv3 = wv[:].rearrange("(a p) d -> p a d", p=128)
    wo3 = wo[:].rearrange("(a p) d -> p a d", p=128)
    mask3 = maskt[:].rearrange("(m p) q -> p m q", p=128)

    with tile.TileContext(nc) as tc:
        with (
            tc.tile_pool(name="const", bufs=1) as cpool,
            tc.tile_pool(name="hid", bufs=3) as hidpool,
            tc.tile_pool(name="kv", bufs=2) as kvpool,
            tc.tile_pool(name="qt", bufs=6) as qtpool,
            tc.tile_pool(name="work", bufs=2) as wpool,
            tc.tile_pool(name="pt", bufs=3) as ptpool,
            tc.tile_pool(name="mm", bufs=3, space="PSUM") as mmpool,
            tc.tile_pool(name="sps", bufs=2, space="PSUM") as spool,
            tc.tile_pool(name="ctxps", bufs=1, space="PSUM") as cxpool,
            tc.tile_pool(name="lps", bufs=1, space="PSUM") as lpool,
            tc.tile_pool(name="dram", bufs=1, space="DRAM") as dpool,
        ):
            # ---- resident constants (single batched DMAs) ----
            wq_sb = cpool.tile([128, NA * HPC * DH], bf, tag="wq")
            wk_sb = cpool.tile([128, NA * DH], bf, tag="wk")
            wv_sb = cpool.tile([128, NA * DH], bf, tag="wv")
            wo_sb = cpool.tile([128, NA * HPC * DH], bf, tag="wo")
            cos_sb = cpool.tile([128, S], f32, tag="cos")
            sin_sb = cpool.tile([128, S], f32, tag="sin")
            mask_sb = cpool.tile([128, len(MASK_RS) * QB], bf, tag="mask")
            ones_sb = cpool.tile([128, 1], bf, tag="ones")

            nc.sync.dma_start(
                wq_sb[:].rearrange("p (a d) -> p a d", a=NA), wq3[:, :, :]
            )
            nc.sync.dma_start(
                wk_sb[:].rearrange("p (a d) -> p a d", a=NA), wk3[:, :, :]
            )
            nc.sync.dma_start(
                wv_sb[:].rearrange("p (a d) -> p a d", a=NA), wv3[:, :, :]
            )
            nc.sync.dma_start(
                wo_sb[:].rearrange("p (a d) -> p a d", a=NA), wo3[:, :, :]
            )
            nc.sync.dma_start(cos_sb[:], cost[:])
            nc.sync.dma_start(sin_sb[:], sinm[:])
            nc.sync.dma_start(
                mask_sb[:].rearrange("p (m q) -> p m q", m=len(MASK_RS)),
                mask3[:, :, :],
            )
            nc.any.memset(ones_sb[:], 1.0)

            # per (batch, tok-half) collective bounce buffers
            ctxl = [
                [
                    dpool.tile(
                        [HPC * DH, S // 2], bf,
                        tag=f"ctxl{b}{hf}", name=f"ctxl{b}{hf}",
                    )
                    for hf in range(2)
                ]
                for b in range(B)
            ]
            ctxf = [
                [
                    dpool.tile(
                        [H * DH, S // 2], bf, addr_space="Shared",
                        tag=f"ctxf{b}{hf}", name=f"ctxf{b}{hf}",
                    )
                    for hf in range(2)
                ]
                for b in range(B)
            ]

            def load_half(src3, gofs, a0, n, width=QB):
                """One DMA: chunks [a0, a0+n) of a (a p)-major DRAM tensor into
                an SBUF tile laid out [128, n*width]."""
                t = hidpool.tile([128, n * width], bf, tag="hid", name=f"hid{gofs}_{a0}")
                nc.sync.dma_start(
                    t[:].rearrange("p (a t) -> p a t", a=n),
                    src3[:, a0 : a0 + n, gofs : gofs + width],
                )
                return t

            def rope_drain(ps, dst, tok0):
                """dst(bf16) = ps * cos + rot_half(ps) * sin (sign-folded)."""
                t1 = wpool.tile([128, QB], f32, tag="ropet1")
                t2 = wpool.tile([128, QB], f32, tag="ropet2")
                cs = cos_sb[:, tok0 : tok0 + QB]
                sn = sin_sb[:, tok0 : tok0 + QB]
                nc.vector.tensor_mul(t1[:], ps, cs)
                nc.vector.tensor_mul(t2[0:64, :], ps[64:128, :], sn[0:64, :])
                nc.vector.tensor_mul(t2[64:128, :], ps[0:64, :], sn[64:128, :])
                nc.vector.tensor_add(dst, t1[:], t2[:])

            def proj_block(b, qbi, kT_sb, v_sb):
                """Projections + RoPE for tokens [qbi*QB, (qbi+1)*QB) of batch b.
                Returns the 4 per-head qT tiles."""
                ltok = qbi * QB
                gtok = b * S + ltok
                halves = [load_half(hidden3, gtok, 0, NA // 2),
                          load_half(hidden3, gtok, NA // 2, NA // 2)]
                qts = [
                    qtpool.tile([128, QB], bf, tag="qtile", name=f"qt{b}_{qbi}_{h}")
                    for h in range(HPC)
                ]
                # group 1: q heads 0..2 ; group 2: q head 3, k, v
                # NOTE: start=True clears has_written for the whole PSUM bank,
                # so regions sharing a bank (v's 4 tok-subtiles) must each run
                # their full accumulation consecutively (j outer, a inner).
                for grp in (("q0", "q1", "q2"), ("q3", "k", "v")):
                    ps = {u: mmpool.tile([128, QB], f32, tag="mmps", name=f"ps{u}{b}{qbi}")
                          for u in grp}
                    for hf in range(2):
                        hs = halves[hf]
                        for u in grp:
                            if u == "v":
                                continue
                            for ai in range(NA // 2):
                                a = hf * (NA // 2) + ai
                                st = a == 0
                                sp = a == NA - 1
                                if u[0] == "q":
                                    h = int(u[1])
                                    nc.tensor.matmul(
                                        ps[u][:],
                                        wq_sb[:, a * 512 + h * 128 : a * 512 + (h + 1) * 128],
                                        hs[:, ai * QB : (ai + 1) * QB],
                                        start=st, stop=sp,
                                    )
                                else:
                                    nc.tensor.matmul(
                                        ps[u][:],
                                        wk_sb[:, a * 128 : (a + 1) * 128],
                                        hs[:, ai * QB : (ai + 1) * QB],
                                        start=st, stop=sp,
                                    )
                        if "v" in grp and hf == 1:
                            for j in range(4):
                                for a in range(NA):
                                    hs2 = halves[a // (NA // 2)]
                                    ai = a % (NA // 2)
                                    nc.tensor.matmul(
                                        ps["v"][:, j * 128 : (j + 1) * 128],
                                        hs2[:, ai * QB + j * 128 : ai * QB + (j + 1) * 128],
                                        wv_sb[:, a * 128 : (a + 1) * 128],
                                        start=(a == 0), stop=(a == NA - 1),
                                    )
                    for u in grp:
                        if u[0] == "q":
                            rope_drain(ps[u][:], qts[int(u[1])][:], ltok)
                        elif u == "k":
                            rope_drain(ps[u][:], kT_sb[:, ltok : ltok + QB], ltok)
                        else:
                            nc.vector.tensor_copy(v_sb[:, ltok : ltok + QB], ps[u][:])
                return qts

            def attn_block(b, qbi, qts, kT_sb, v_sb):
                Q0 = 4 * qbi
                kts = [Q0] + [kt for kt in range(max(0, Q0 - 8), Q0 + 4) if kt != Q0]
                for h in range(HPC):
                    qt = qts[h]
                    ctx_ps = cxpool.tile([128, QB], f32, tag="ctxps", name=f"cx{b}{qbi}{h}")
                    l_ps = lpool.tile([1, QB], f32, tag="lps", name=f"l{b}{qbi}{h}")
                    for idx, kt in enumerate(kts):
                        r = kt - (Q0 - 8)
                        qlo, qhi = _span(r)
                        s_ps = spool.tile([128, QB], f32, tag="sps", name=f"s{b}{qbi}{h}{kt}")
                        nc.tensor.matmul(
                            s_ps[:, qlo:qhi],
                            kT_sb[:, kt * 128 : (kt + 1) * 128],
                            qt[:, qlo:qhi],
                            start=True, stop=True,
                        )
                        pt = ptpool.tile([128, QB], bf, tag="pt", name=f"pt{b}{qbi}{h}{kt}")
                        nc.scalar.activation(
                            pt[:, qlo:qhi], s_ps[:, qlo:qhi], AF.Exp, scale=SCALE
                        )
                        if r in MSLOT:
                            m0 = MSLOT[r] * QB
                            nc.vector.tensor_mul(
                                pt[:, qlo:qhi],
                                pt[:, qlo:qhi],
                                mask_sb[:, m0 + qlo : m0 + qhi],
                            )
                        last = idx == len(kts) - 1
                        nc.tensor.matmul(
                            ctx_ps[:, qlo:qhi],
                            v_sb[:, kt * 128 : (kt + 1) * 128],
                            pt[:, qlo:qhi],
                            start=(idx == 0), stop=last,
                        )
                        nc.tensor.matmul(
                            l_ps[0:1, qlo:qhi],
                            ones_sb[:, 0:1],
                            pt[:, qlo:qhi],
                            start=(idx == 0), stop=last,
                        )
                    lrec = wpool.tile([1, QB], f32, tag="lrec", name=f"lr{b}{qbi}{h}")
                    nc.vector.reciprocal(lrec[:], l_ps[:])
                    lb = wpool.tile([128, QB], f32, tag="lb", name=f"lb{b}{qbi}{h}")
                    nc.gpsimd.partition_broadcast(lb[:], lrec[0:1, :])
                    ctx_sb = wpool.tile([128, QB], bf, tag="ctxsb", name=f"cs{b}{qbi}{h}")
                    nc.vector.tensor_mul(ctx_sb[:], ctx_ps[:], lb[:])
                    if b == 0 and qbi == 0 and h == 0:
                        nc.sync.dma_start(dbg[384:512, :], ctx_sb[:])
                        lbd = wpool.tile([128, QB], bf, tag="lbd", name="lbd")
                        nc.vector.tensor_copy(lbd[:], lb[:])
                        nc.sync.dma_start(dbg[512:640, :], lbd[:])
                    nc.sync.dma_start(
                        ctxl[b][qbi // 2][
                            h * 128 : (h + 1) * 128,
                            (qbi % 2) * QB : (qbi % 2 + 1) * QB,
                        ],
                        ctx_sb[:],
                    )

            def allgather(b, hf):
                nc.gpsimd.collective_compute(
                    "AllGather",
                    __import__("concourse.mybir", fromlist=["AluOpType"]).AluOpType.bypass,
                    replica_groups=[list(range(NCORES))],
                    ins=[ctxl[b][hf][:].opt()],
                    outs=[ctxf[b][hf][:].opt()],
                )

            def oproj_block(b, tbo):
                """out^T[oc, tok] for tokens [tbo*QB, +QB) of batch b."""
                ltok = tbo * QB
                gtok = b * S + ltok
                src3 = ctxf[b][tbo // 2][:].rearrange("(a p) t -> p a t", p=128)
                lofs = (tbo % 2) * QB
                cfs = []
                for hf in range(2):
                    t = hidpool.tile(
                        [128, (NA // 2) * QB], bf, tag="hid", name=f"cf{b}{tbo}{hf}"
                    )
                    nc.sync.dma_start(
                        t[:].rearrange("p (a t) -> p a t", a=NA // 2),
                        src3[:, hf * (NA // 2) : (hf + 1) * (NA // 2), lofs : lofs + QB],
                    )
                    cfs.append(t)
                for oc in range(HPC):
                    ps = mmpool.tile([128, QB], f32, tag="mmps", name=f"ops{b}{tbo}{oc}")
                    for a in range(NA):
                        nc.tensor.matmul(
                            ps[:],
                            wo_sb[:, a * 512 + oc * 128 : a * 512 + (oc + 1) * 128],
                            cfs[a // (NA // 2)][:, (a % (NA // 2)) * QB : (a % (NA // 2) + 1) * QB],
                            start=(a == 0), stop=(a == NA - 1),
                        )
                    osb = wpool.tile([128, QB], f32, tag="osb", name=f"ob{b}{tbo}{oc}")
                    nc.vector.tensor_copy(osb[:], ps[:])
                    nc.sync.dma_start(
                        outT[oc * 128 : (oc + 1) * 128, gtok : gtok + QB], osb[:]
                    )

            # ================= emission schedule =================
            for b in range(B):
                kT_sb = kvpool.tile([128, S], bf, tag="kT", name=f"kT{b}")
                v_sb = kvpool.tile([128, S], bf, tag="v", name=f"v{b}")
                for qbi in range(NQB):
                    qts = proj_block(b, qbi, kT_sb, v_sb)
                    if b == 0 and qbi == 0:
                        nc.sync.dma_start(dbg[0:128, :], qts[0][:])
                        nc.sync.dma_start(dbg[128:256, :], kT_sb[:, 0:QB])
                        nc.sync.dma_start(dbg[256:384, :], v_sb[:, 0:QB])
                    attn_block(b, qbi, qts, kT_sb, v_sb)
                    if qbi == 1:
                        allgather(b, 0)
                    if b == 1 and qbi >= 2:
                        oproj_block(0, qbi)  # overlap b0 o_proj with b1 tail
                allgather(b, 1)
            oproj_block(0, 0)
            oproj_block(0, 1)
            for tbo in range(NQB):
                oproj_block(1, tbo)

    nc.compile()
    return nc


_NC = None


def _get_nc():
    global _NC
    if _NC is None:
        _NC = _build_nc()
    return _NC


def _prep_inputs(hidden_states, q_proj_w, k_proj_w, v_proj_w, o_proj_w, position_ids):
    hidden_states = np.asarray(hidden_states, dtype=np.float32)
    hT = np.ascontiguousarray(hidden_states.reshape(BT, HID).T).astype(BF16)

    pos = np.asarray(position_ids)[0].astype(np.float32)  # [S]
    inv = 1.0 / (THETA ** (np.arange(0, DH, 2, dtype=np.float32) / DH))  # [64]
    ang = pos[:, None] * inv[None, :]  # [S, 64]
    c = np.cos(ang).T.astype(np.float32)  # [64, S]
    s = np.sin(ang).T.astype(np.float32)
    cost = np.ascontiguousarray(np.concatenate([c, c], axis=0))
    sinm = np.ascontiguousarray(np.concatenate([-s, s], axis=0))

    kj = np.arange(128)[:, None]
    qi = np.arange(QB)[None, :]
    masks = []
    for r in MASK_RS:
        d = (8 - r) * 128 + qi - kj
        masks.append(((d >= 0) & (d < WIN)).astype(np.float32))
    maskt = np.ascontiguousarray(np.concatenate(masks, axis=0)).astype(BF16)

    q_proj_w = np.asarray(q_proj_w, dtype=np.float32)
    k_proj_w = np.asarray(k_proj_w, dtype=np.float32)
    v_proj_w = np.asarray(v_proj_w, dtype=np.float32)
    o_proj_w = np.asarray(o_proj_w, dtype=np.float32)

    in_maps = []
    for core in range(NCORES):
        r0q = core * HPC * DH
        r0k = core * DH
        in_maps.append(
            {
                "hiddenT": hT,
                "wq": np.ascontiguousarray(
                    q_proj_w[r0q : r0q + HPC * DH, :].T
                ).astype(BF16),
                "wk": np.ascontiguousarray(k_proj_w[r0k : r0k + DH, :].T).astype(BF16),
                "wv": np.ascontiguousarray(v_proj_w[r0k : r0k + DH, :].T).astype(BF16),
                "wo": np.ascontiguousarray(
                    o_proj_w[r0q : r0q + HPC * DH, :].T
                ).astype(BF16),
                "cost": cost,
                "sinm": sinm,
                "maskt": maskt,
            }
        )
    return in_maps


def run(inputs, trace=False):
    from concourse.bass_utils import run_bass_kernel_spmd

    nc = _get_nc()
    in_maps = _prep_inputs(
        inputs["hidden_states"],
        inputs["q_proj_w"],
        inputs["k_proj_w"],
        inputs["v_proj_w"],
        inputs["o_proj_w"],
        inputs["position_ids"],
    )
    res = run_bass_kernel_spmd(
        nc, in_maps, core_ids=list(range(NCORES)), trace=trace
    )
    out = np.empty((BT, HID), dtype=np.float32)
    for core in range(NCORES):
        o = np.asarray(res.results[core]["outT"], dtype=np.float32)  # [512, BT]
        out[:, core * HPC * DH : (core + 1) * HPC * DH] = o.T
    return out.reshape(B, S, HID), res


def kernel(**inputs):
    out, _ = run(inputs, trace=False)
    return out


# revision 21
# speedup vs baseline: 1.2719x; 1.0748x over previous
"""Distributed Trainium2 Bass kernel for ArcticAttention (GQA + RoPE + sliding window).

Sharding: tensor-parallel over heads across 8 cores. Core c owns q heads
4c..4c+3 and kv head c (exactly one GQA group). Per core:
  - q/k/v projections (bf16 matmuls, fp32 PSUM) producing qT/kT [dh, tok]
    and v [tok, dh] layouts,
  - RoPE fused on the vector engine from host-precomputed cos/sign-folded-sin
    tables,
  - sliding-window attention in S^T = K@Q^T layout (softmax over the
    partition axis via a ones-vector matmul; 1/l broadcast via gpsimd
    partition_broadcast; PV matmul needs no transposes anywhere),
  - AllGather of ctx^T features (bf16, 1 MB/core per batch-half, 4 total,
    interleaved with compute so the wire time hides),
  - column-sharded o_proj producing out^T [oc, tok]; host concatenates.
"""

import os
import sys

sys.path.insert(0, "/opt/pypackages")
sys.path.insert(0, "/opt/trn_rl_repo")

import numpy as np
import ml_dtypes

BF16 = ml_dtypes.bfloat16

B, S, HID = 2, 2048, 4096
H, HKV, DH = 32, 8, 128
G = H // HKV
WIN = 1024
THETA = 10000.0
NCORES = 8
HPC = H // NCORES          # 4 q heads per core
BT = B * S                 # 4096 tokens
QB = 512                   # token block for projections, attention, o_proj
NQB = S // QB              # 4 blocks per batch
NA = HID // 128            # 32 hid chunks
SCALE = 1.0 / float(np.sqrt(DH))

MASK_RS = (0, 1, 2, 3, 8, 9, 10, 11)
MSLOT = {r: i for i, r in enumerate(MASK_RS)}


def _span(r):
    qlo = max(0, (r - 8) * 128)
    qhi = min(QB, (r - 8) * 128 + 1024 + 127)
    return qlo, qhi


def _build_nc():
    import concourse.bass as bass
    import concourse.bacc as bacc
    import concourse.mybir as mybir
    from concourse import tile

    dt = mybir.dt
    bf = dt.bfloat16
    f32 = dt.float32
    AF = mybir.ActivationFunctionType

    nc = bacc.Bacc(
        "TRN2",
        target_bir_lowering=False,
        debug=False,
        enable_asserts=False,
        num_devices=NCORES,
    )

    hiddenT = nc.dram_tensor("hiddenT", [HID, BT], bf, kind="ExternalInput")
    wq = nc.dram_tensor("wq", [HID, HPC * DH], bf, kind="ExternalInput")
    wk = nc.dram_tensor("wk", [HID, DH], bf, kind="ExternalInput")
    wv = nc.dram_tensor("wv", [HID, DH], bf, kind="ExternalInput")
    wo = nc.dram_tensor("wo", [HID, HPC * DH], bf, kind="ExternalInput")
    cost = nc.dram_tensor("cost", [DH, S], f32, kind="ExternalInput")
    sinm = nc.dram_tensor("sinm", [DH, S], f32, kind="ExternalInput")
    maskt = nc.dram_tensor("maskt", [len(MASK_RS) * 128, QB], bf, kind="ExternalInput")
    outT = nc.dram_tensor("outT", [HPC * DH, BT], f32, kind="ExternalOutput")

    hidden3 = hiddenT[:].rearrange("(a p) t -> p a t", p=128)
    wq3 = wq[:].rearrange("(a p) d -> p a d", p=128)
    wk3 = wk[:].rearrange("(a p) d -> p a d", p=128)
    wv3 = wv[:].rearrange("(a p) d -> p a d", p=128)
    wo3 = wo[:].rearrange("(a p) d -> p a d", p=128)
    mask3 = maskt[:].rearrange("(m p) q -> p m q", p=128)

    with tile.TileContext(nc) as tc:
        with (
            tc.tile_pool(name="const", bufs=1) as cpool,
            tc.tile_pool(name="hid", bufs=3) as hidpool,
            tc.tile_pool(name="kv", bufs=2) as kvpool,
            tc.tile_pool(name="qt", bufs=6) as qtpool,
            tc.tile_pool(name="work", bufs=2) as wpool,
            tc.tile_pool(name="pt", bufs=3) as ptpool,
            tc.tile_pool(name="mm", bufs=3, space="PSUM") as mmpool,
            tc.tile_pool(name="sps", bufs=2, space="PSUM") as spool,
            tc.tile_pool(name="ctxps", bufs=1, space="PSUM") as cxpool,
            tc.tile_pool(name="lps", bufs=1, space="PSUM") as lpool,
            tc.tile_pool(name="dram", bufs=1, space="DRAM") as dpool,
        ):
            # ---- resident constants (single batched DMAs) ----
            wq_sb = cpool.tile([128, NA * HPC * DH], bf, tag="wq")
            wk_sb = cpool.tile([128, NA * DH], bf, tag="wk")
            wv_sb = cpool.tile([128, NA * DH], bf, tag="wv")
            wo_sb = cpool.tile([128, NA * HPC * DH], bf, tag="wo")
            cos_sb = cpool.tile([128, S], f32, tag="cos")
            sin_sb = cpool.tile([128, S], f32, tag="sin")
            mask_sb = cpool.tile([128, len(MASK_RS) * QB], bf, tag="mask")
            ones_sb = cpool.tile([128, 1], bf, tag="ones")

            nc.sync.dma_start(
                wq_sb[:].rearrange("p (a d) -> p a d", a=NA), wq3[:, :, :]
            )
            nc.sync.dma_start(
                wk_sb[:].rearrange("p (a d) -> p a d", a=NA), wk3[:, :, :]
            )
            nc.sync.dma_start(
                wv_sb[:].rearrange("p (a d) -> p a d", a=NA), wv3[:, :, :]
            )
            nc.sync.dma_start(
                wo_sb[:].rearrange("p (a d) -> p a d", a=NA), wo3[:, :, :]
            )
            nc.sync.dma_start(cos_sb[:], cost[:])
            nc.sync.dma_start(sin_sb[:], sinm[:])
            nc.sync.dma_start(
                mask_sb[:].rearrange("p (m q) -> p m q", m=len(MASK_RS)),
                mask3[:, :, :],
            )
            nc.any.memset(ones_sb[:], 1.0)

            # per (batch, tok-half) collective bounce buffers
            ctxl = [
                [
                    dpool.tile(
                        [HPC * DH, S // 2], bf,
                        tag=f"ctxl{b}{hf}", name=f"ctxl{b}{hf}",
                    )
                    for hf in range(2)
                ]
                for b in range(B)
            ]
            ctxf = [
                [
                    dpool.tile(
                        [H * DH, S // 2], bf, addr_space="Shared",
                        tag=f"ctxf{b}{hf}", name=f"ctxf{b}{hf}",
                    )
                    for hf in range(2)
                ]
                for b in range(B)
            ]

            def load_half(src3, gofs, a0, n, width=QB):
                """One DMA: chunks [a0, a0+n) of a (a p)-major DRAM tensor into
                an SBUF tile laid out [128, n*width]."""
                t = hidpool.tile([128, n * width], bf, tag="hid", name=f"hid{gofs}_{a0}")
                nc.sync.dma_start(
                    t[:].rearrange("p (a t) -> p a t", a=n),
                    src3[:, a0 : a0 + n, gofs : gofs + width],
                )
                return t

            def rope_drain(ps, dst, tok0):
                """dst(bf16) = ps * cos + rot_half(ps) * sin (sign-folded)."""
                t1 = wpool.tile([128, QB], f32, tag="ropet1")
                t2 = wpool.tile([128, QB], f32, tag="ropet2")
                cs = cos_sb[:, tok0 : tok0 + QB]
                sn = sin_sb[:, tok0 : tok0 + QB]
                nc.vector.tensor_mul(t1[:], ps, cs)
                nc.vector.tensor_mul(t2[0:64, :], ps[64:128, :], sn[0:64, :])
                nc.vector.tensor_mul(t2[64:128, :], ps[0:64, :], sn[64:128, :])
                nc.vector.tensor_add(dst, t1[:], t2[:])

            def proj_block(b, qbi, kT_sb, v_sb):
                """Projections + RoPE for tokens [qbi*QB, (qbi+1)*QB) of batch b.
                Returns the 4 per-head qT tiles."""
                ltok = qbi * QB
                gtok = b * S + ltok
                halves = [load_half(hidden3, gtok, 0, NA // 2),
                          load_half(hidden3, gtok, NA // 2, NA // 2)]
                qts = [
                    qtpool.tile([128, QB], bf, tag="qtile", name=f"qt{b}_{qbi}_{h}")
                    for h in range(HPC)
                ]
                # group 1: q heads 0..2 ; group 2: q head 3, k, v
                # NOTE: start=True clears has_written for the whole PSUM bank,
                # so regions sharing a bank (v's 4 tok-subtiles) must each run
                # their full accumulation consecutively (j outer, a inner).
                for grp in (("q0", "q1", "q2"), ("q3", "k", "v")):
                    ps = {u: mmpool.tile([128, QB], f32, tag="mmps", name=f"ps{u}{b}{qbi}")
                          for u in grp}
                    for hf in range(2):
                        hs = halves[hf]
                        for u in grp:
                            if u == "v":
                                continue
                            for ai in range(NA // 2):
                                a = hf * (NA // 2) + ai
                                st = a == 0
                                sp = a == NA - 1
                                if u[0] == "q":
                                    h = int(u[1])
                                    nc.tensor.matmul(
                                        ps[u][:],
                                        wq_sb[:, a * 512 + h * 128 : a * 512 + (h + 1) * 128],
                                        hs[:, ai * QB : (ai + 1) * QB],
                                        start=st, stop=sp,
                                    )
                                else:
                                    nc.tensor.matmul(
                                        ps[u][:],
                                        wk_sb[:, a * 128 : (a + 1) * 128],
                                        hs[:, ai * QB : (ai + 1) * QB],
                                        start=st, stop=sp,
                                    )
                        if "v" in grp and hf == 1:
                            for j in range(4):
                                for a in range(NA):
                                    hs2 = halves[a // (NA // 2)]
                                    ai = a % (NA // 2)
                                    nc.tensor.matmul(
                                        ps["v"][:, j * 128 : (j + 1) * 128],
                                        hs2[:, ai * QB + j * 128 : ai * QB + (j + 1) * 128],
                                        wv_sb[:, a * 128 : (a + 1) * 128],
                                        start=(a == 0), stop=(a == NA - 1),
                                    )
                    for u in grp:
                        if u[0] == "q":
                            rope_drain(ps[u][:], qts[int(u[1])][:], ltok)
                        elif u == "k":
                            rope_drain(ps[u][:], kT_sb[:, ltok : ltok + QB], ltok)
                        else:
                            nc.vector.tensor_copy(v_sb[:, ltok : ltok + QB], ps[u][:])
                return qts

            def attn_block(b, qbi, qts, kT_sb, v_sb):
                Q0 = 4 * qbi
                kts = [Q0] + [kt for kt in range(max(0, Q0 - 8), Q0 + 4) if kt != Q0]
                for h in range(HPC):
                    qt = qts[h]
                    ctx_ps = cxpool.tile([128, QB], f32, tag="ctxps", name=f"cx{b}{qbi}{h}")
                    l_ps = lpool.tile([1, QB], f32, tag="lps", name=f"l{b}{qbi}{h}")
                    for idx, kt in enumerate(kts):
                        r = kt - (Q0 - 8)
                        qlo, qhi = _span(r)
                        s_ps = spool.tile([128, QB], f32, tag="sps", name=f"s{b}{qbi}{h}{kt}")
                        nc.tensor.matmul(
                            s_ps[:, qlo:qhi],
                            kT_sb[:, kt * 128 : (kt + 1) * 128],
                            qt[:, qlo:qhi],
                            start=True, stop=True,
                        )
                        pt = ptpool.tile([128, QB], bf, tag="pt", name=f"pt{b}{qbi}{h}{kt}")
                        nc.scalar.activation(
                            pt[:, qlo:qhi], s_ps[:, qlo:qhi], AF.Exp, scale=SCALE
                        )
                        if r in MSLOT:
                            m0 = MSLOT[r] * QB
                            nc.vector.tensor_mul(
                                pt[:, qlo:qhi],
                                pt[:, qlo:qhi],
                                mask_sb[:, m0 + qlo : m0 + qhi],
                            )
                        last = idx == len(kts) - 1
                        nc.tensor.matmul(
                            ctx_ps[:, qlo:qhi],
                            v_sb[:, kt * 128 : (kt + 1) * 128],
                            pt[:, qlo:qhi],
                            start=(idx == 0), stop=last,
                        )
                        nc.tensor.matmul(
                            l_ps[0:1, qlo:qhi],
                            ones_sb[:, 0:1],
                            pt[:, qlo:qhi],
                            start=(idx == 0), stop=last,
                        )
                    lrec = wpool.tile([1, QB], f32, tag="lrec", name=f"lr{b}{qbi}{h}")
                    nc.vector.reciprocal_approx_fast(lrec[:], l_ps[:])
                    lb = wpool.tile([128, QB], f32, tag="lb", name=f"lb{b}{qbi}{h}")
                    nc.gpsimd.partition_broadcast(lb[:], lrec[0:1, :])
                    ctx_sb = wpool.tile([128, QB], bf, tag="ctxsb", name=f"cs{b}{qbi}{h}")
                    nc.vector.tensor_mul(ctx_sb[:], ctx_ps[:], lb[:])

                    nc.sync.dma_start(
                        ctxl[b][qbi // 2][
                            h * 128 : (h + 1) * 128,
                            (qbi % 2) * QB : (qbi % 2 + 1) * QB,
                        ],
                        ctx_sb[:],
                    )

            def allgather(b, hf):
                nc.gpsimd.collective_compute(
                    "AllGather",
                    __import__("concourse.mybir", fromlist=["AluOpType"]).AluOpType.bypass,
                    replica_groups=[list(range(NCORES))],
                    ins=[ctxl[b][hf][:].opt()],
                    outs=[ctxf[b][hf][:].opt()],
                )

            def oproj_block(b, tbo):
                """out^T[oc, tok] for tokens [tbo*QB, +QB) of batch b."""
                ltok = tbo * QB
                gtok = b * S + ltok
                src3 = ctxf[b][tbo // 2][:].rearrange("(a p) t -> p a t", p=128)
                lofs = (tbo % 2) * QB
                cfs = []
                for hf in range(2):
                    t = hidpool.tile(
                        [128, (NA // 2) * QB], bf, tag="hid", name=f"cf{b}{tbo}{hf}"
                    )
                    nc.sync.dma_start(
                        t[:].rearrange("p (a t) -> p a t", a=NA // 2),
                        src3[:, hf * (NA // 2) : (hf + 1) * (NA // 2), lofs : lofs + QB],
                    )
                    cfs.append(t)
                for oc in range(HPC):
                    ps = mmpool.tile([128, QB], f32, tag="mmps", name=f"ops{b}{tbo}{oc}")
                    for a in range(NA):
                        nc.tensor.matmul(
                            ps[:],
                            wo_sb[:, a * 512 + oc * 128 : a * 512 + (oc + 1) * 128],
                            cfs[a // (NA // 2)][:, (a % (NA // 2)) * QB : (a % (NA // 2) + 1) * QB],
                            start=(a == 0), stop=(a == NA - 1),
                        )
                    osb = wpool.tile([128, QB], f32, tag="osb", name=f"ob{b}{tbo}{oc}")
                    nc.vector.tensor_copy(osb[:], ps[:])
                    nc.sync.dma_start(
                        outT[oc * 128 : (oc + 1) * 128, gtok : gtok + QB], osb[:]
                    )

            # ================= emission schedule =================
            for b in range(B):
                kT_sb = kvpool.tile([128, S], bf, tag="kT", name=f"kT{b}")
                v_sb = kvpool.tile([128, S], bf, tag="v", name=f"v{b}")
                for qbi in range(NQB):
                    qts = proj_block(b, qbi, kT_sb, v_sb)
                    attn_block(b, qbi, qts, kT_sb, v_sb)
                    if qbi == 1:
                        allgather(b, 0)
                    if b == 1 and qbi >= 2:
                        oproj_block(0, qbi)  # overlap b0 o_proj with b1 tail
                allgather(b, 1)
            oproj_block(0, 0)
            oproj_block(0, 1)
            for tbo in range(NQB):
                oproj_block(1, tbo)

    nc.compile()
    return nc


_NC = None


def _get_nc():
    global _NC
    if _NC is None:
        _NC = _build_nc()
    return _NC


def _prep_inputs(hidden_states, q_proj_w, k_proj_w, v_proj_w, o_proj_w, position_ids):
    hidden_states = np.asarray(hidden_states, dtype=np.float32)
    hT = np.ascontiguousarray(hidden_states.reshape(BT, HID).T).astype(BF16)

    pos = np.asarray(position_ids)[0].astype(np.float32)  # [S]
    inv = 1.0 / (THETA ** (np.arange(0, DH, 2, dtype=np.float32) / DH))  # [64]
    ang = pos[:, None] * inv[None, :]  # [S, 64]
    c = np.cos(ang).T.astype(np.float32)  # [64, S]
    s = np.sin(ang).T.astype(np.float32)
    cost = np.ascontiguousarray(np.concatenate([c, c], axis=0))
    sinm = np.ascontiguousarray(np.concatenate([-s, s], axis=0))

    kj = np.arange(128)[:, None]
    qi = np.arange(QB)[None, :]
    masks = []
    for r in MASK_RS:
        d = (8 - r) * 128 + qi - kj
        masks.append(((d >= 0) & (d < WIN)).astype(np.float32))
    maskt = np.ascontiguousarray(np.concatenate(masks, axis=0)).astype(BF16)

    q_proj_w = np.asarray(q_proj_w, dtype=np.float32)
    k_proj_w = np.asarray(k_proj_w, dtype=np.float32)
    v_proj_w = np.asarray(v_proj_w, dtype=np.float32)
    o_proj_w = np.asarray(o_proj_w, dtype=np.float32)

    in_maps = []
    for core in range(NCORES):
        r0q = core * HPC * DH
        r0k = core * DH
        in_maps.append(
            {
                "hiddenT": hT,
                "wq": np.ascontiguousarray(
                    q_proj_w[r0q : r0q + HPC * DH, :].T
                ).astype(BF16),
                "wk": np.ascontiguousarray(k_proj_w[r0k : r0k + DH, :].T).astype(BF16),
                "wv": np.ascontiguousarray(v_proj_w[r0k : r0k + DH, :].T).astype(BF16),
                "wo": np.ascontiguousarray(
                    o_proj_w[r0q : r0q + HPC * DH, :].T
                ).astype(BF16),
                "cost": cost,
                "sinm": sinm,
                "maskt": maskt,
            }
        )
    return in_maps


def run(inputs, trace=False):
    from concourse.bass_utils import run_bass_kernel_spmd

    nc = _get_nc()
    in_maps = _prep_inputs(
        inputs["hidden_states"],
        inputs["q_proj_w"],
        inputs["k_proj_w"],
        inputs["v_proj_w"],
        inputs["o_proj_w"],
        inputs["position_ids"],
    )
    res = run_bass_kernel_spmd(
        nc, in_maps, core_ids=list(range(NCORES)), trace=trace
    )
    out = np.empty((BT, HID), dtype=np.float32)
    for core in range(NCORES):
        o = np.asarray(res.results[core]["outT"], dtype=np.float32)  # [512, BT]
        out[:, core * HPC * DH : (core + 1) * HPC * DH] = o.T
    return out.reshape(B, S, HID), res


def kernel(**inputs):
    out, _ = run(inputs, trace=False)
    return out
